# revision 14
# baseline (speedup 1.0000x reference)
"""LIF (leaky integrate-and-fire) forward scan on 8 Trainium2 NeuronCores.

Reference recurrence (per element, scan over T):
    m_t = v_{t-1} * tau + x_t
    y_t = (m_t - v_th > 0) ? 1.0 : 0.0
    v_t = m_t * (1 - y_t)          # hard reset on spike

x: [T=16, B=32, C=128, H=32, W=32] f32.  Data-parallel over B: each core
gets B_loc=4 batches. Host pre-transposes the per-core block to
[T, C, F=4*H*W] so every per-step DMA is one fully-contiguous
16KiB-per-partition transfer.

Per step (F=4096 sites, two 2048-col chunks whose chains interleave on DVE):
  DVE: m  = scalar_tensor_tensor(v, tau, x, mult, add)     (skipped at t=0)
       v' = scalar_tensor_tensor(m, v_th, m, is_le, mult)  (skipped at t=T-1)
  ACT: y  = Sign(m - v_th) -> uint8 in ONE op (f32->u8 saturating convert
       maps -1 -> 0, so the result is exactly (m > v_th)); host -> f32.
  DMA: x loads on the sync HWDGE ring, y stores on the scalar ring.
DVE is the bottleneck (~133 us busy; 2 ops/elem is its f32 I/O floor —
STT reads 2 tensors at 1 elem/cyc); everything else overlaps. Measured
~154.6 us on HW with bit-exact output vs the jax reference.
"""

import sys

sys.path.insert(0, "/opt/trn_rl_repo")

from contextlib import ExitStack

import numpy as np

import concourse.bass as bass
import concourse.tile as tile
from concourse import bacc, mybir
from concourse.bass_utils import run_bass_kernel_spmd

# Hyperparameters (from the nn.Module)
V_TH = 1.0
TAU = 0.5

# Shapes (hardcoded per problem spec)
T, B, C, H, W = 16, 32, 128, 32, 32
N_CORES = 8
B_LOC = B // N_CORES           # 4 batches per core
S = H * W                      # 1024 spatial sites
F = B_LOC * S                  # 4096 free-dim sites per step

DT = mybir.dt.float32
U8 = mybir.dt.uint8


def build_kernel() -> bass.Bass:
    nc = bacc.Bacc(
        "TRN2", target_bir_lowering=False, debug=False, num_devices=N_CORES
    )
    x_d = nc.dram_tensor("x", [T, C, F], DT, kind="ExternalInput").ap()
    y_d = nc.dram_tensor("y", [T, C, F], U8, kind="ExternalOutput").ap()

    # Register a -V_TH const AP (activation bias needs a [128,1] SBUF const).
    _c = nc.alloc_sbuf_tensor(f"const-float32-{-V_TH}", [128, 1], DT)
    nc.gpsimd.memset(_c.ap(), -V_TH)
    nc.const_aps.aps[(DT, -V_TH)] = _c.ap()
    nc.all_engine_barrier()

    with ExitStack() as ctx:
        tc = ctx.enter_context(tile.TileContext(nc))
        x_pool = ctx.enter_context(tc.tile_pool(name="x", bufs=5))
        m_pool = ctx.enter_context(tc.tile_pool(name="m", bufs=3))
        v_pool = ctx.enter_context(tc.tile_pool(name="v", bufs=2))
        y_pool = ctx.enter_context(tc.tile_pool(name="y", bufs=3))

        # Hybrid chunking: 2 column-chunks at the pipeline boundaries (ramp
        # t=0..2, tail t=14..15) so DVE/ACT/DMA interleave while the pipe
        # fills/drains; a single 4096-col chunk in the middle halves the
        # per-op dispatch overhead. All tiles are full [C, F]; chunked steps
        # just write slices, so phase transitions need no data movement.
        def chunks(t):
            if t == 0:
                return [(0, 512), (512, 2048), (2048, F)]   # graded ramp
            if t in (1, 2, 14, 15):
                return [(0, F // 2), (F // 2, F)]
            return [(0, F)]

        v = None
        for t in range(T):
            xt = x_pool.tile([C, F], DT, tag="x")
            for i, (c0, c1) in enumerate(chunks(t)):
                # At t=0 the scalar ring is still free: alternate rings so
                # the graded sub-loads land in parallel.
                eng = nc.scalar if (t == 0 and i % 2 == 1) else nc.sync
                eng.dma_start(out=xt[:, c0:c1], in_=x_d[t, :, c0:c1])

            # m chunks first, then y / v' chunks — keeps the DVE queue
            # (m..., v'...) stall-free across steps.
            if t == 0:
                m = xt[:]
            else:
                mt = m_pool.tile([C, F], DT, tag="m")
                for (c0, c1) in chunks(t):
                    nc.vector.scalar_tensor_tensor(
                        mt[:, c0:c1], v[:, c0:c1], TAU, xt[:, c0:c1],
                        mybir.AluOpType.mult, mybir.AluOpType.add,
                    )
                m = mt[:]

            yt = y_pool.tile([C, F], U8, tag="y")
            vt = (
                v_pool.tile([C, F], DT, tag="v", name="v")
                if t < T - 1
                else None
            )
            for (c0, c1) in chunks(t):
                nc.scalar.activation(
                    yt[:, c0:c1], m[:, c0:c1],
                    mybir.ActivationFunctionType.Sign, bias=-V_TH,
                )
                if vt is not None:
                    nc.vector.scalar_tensor_tensor(
                        vt[:, c0:c1], m[:, c0:c1], V_TH, m[:, c0:c1],
                        mybir.AluOpType.is_le, mybir.AluOpType.mult,
                    )
                else:
                    # Last step: store per chunk so the final store overlaps
                    # the other chunk's activation.
                    nc.scalar.dma_start(out=y_d[t, :, c0:c1], in_=yt[:, c0:c1])
            if vt is not None:
                nc.scalar.dma_start(out=y_d[t], in_=yt[:])
                v = vt
    nc.finalize()
    return nc


_NC_CACHE = None


def _get_nc():
    global _NC_CACHE
    if _NC_CACHE is None:
        _NC_CACHE = build_kernel()
    return _NC_CACHE


def _in_maps(x: np.ndarray) -> list[dict]:
    xf = np.asarray(x, dtype=np.float32).reshape(T, B, C, S)
    maps = []
    for k in range(N_CORES):
        blk = xf[:, k * B_LOC:(k + 1) * B_LOC]          # [T, B_loc, C, S]
        blk = np.ascontiguousarray(blk.transpose(0, 2, 1, 3))  # [T, C, B_loc, S]
        maps.append({"x": blk.reshape(T, C, F)})
    return maps


def kernel(x: np.ndarray) -> np.ndarray:
    assert x.shape == (T, B, C, H, W), x.shape
    in_dtype = x.dtype
    nc = _get_nc()
    in_maps = _in_maps(x)
    res = run_bass_kernel_spmd(nc, in_maps, list(range(N_CORES)))
    parts = []
    for k in range(N_CORES):
        yk = res.results[k]["y"].reshape(T, C, B_LOC, S).transpose(0, 2, 1, 3)
        parts.append(yk)                                # [T, B_loc, C, S]
    out = np.concatenate(parts, axis=1)                 # [T, B, C, S]
    return out.reshape(T, B, C, H, W).astype(in_dtype, copy=False)


if __name__ == "__main__":
    x = np.random.randn(T, B, C, H, W).astype(np.float32)
    y = kernel(x)
    print("out", y.shape, y.dtype, "spike rate", y.mean())
